# revision 9
# baseline (speedup 1.0000x reference)
"""Trainium2 Bass kernel for nn_Decoder: Bahdanau attention + 4-layer LSTM
(single step, zero initial state) + vocab projection, on 8 NeuronCores.

Sharding: attention is data-parallel over batch (8 batches/core); the LSTM
stack is tensor-parallel over gate columns (128 of 1024 h-features per core,
only i/g/o gates — the f gate multiplies zero state and Wr multiplies h0=0);
the fc vocab projection is tensor-parallel over vocab (4000 cols/core).
Feature-major (transposed) activations flow through the LSTM so every matmul
has its contraction dim on partitions; 5 small AllGathers (context + 4 h's)
stitch the cores together.  All matmuls run in float32r (~1e-4 rel err).
"""
import numpy as np

import concourse.bass as bass
import concourse.mybir as mybir
import concourse.tile as tile
from concourse import bacc
from concourse.bass_utils import run_bass_kernel_spmd
from concourse.masks import make_identity

P = 128
NCORES = 8
B, S, H, EMB, VOCAB = 64, 128, 1024, 1024, 32000
BL = B // NCORES          # 8 batches per core
HL = H // NCORES          # 128 gate columns / h features per core
VL = VOCAB // NCORES      # 4000 vocab columns per core
KT = H // P               # 8 k-tiles over a 1024 feature dim
NT = 8                    # fc n-tiles
NW = VL // NT             # 500

F32 = mybir.dt.float32
F32R = mybir.dt.float32r
I32 = mybir.dt.int32
AF = mybir.ActivationFunctionType
ALU = mybir.AluOpType
AX = mybir.AxisListType

FCW_BUFS = 4   # fcW k-tile slots resident (of 8 tiles, 2 MB each)
RG = [list(range(NCORES))]


def _build():
    nc = bacc.Bacc(
        "TRN2", target_bir_lowering=False, debug=False, num_devices=NCORES
    )

    # ---- per-core inputs (host pre-shards / pre-slices) ----
    d = {}

    def inp(name, shape, dt):
        d[name] = nc.dram_tensor(name, shape, dt, kind="ExternalInput").ap()

    inp("x_idx", [B, 1], I32)
    inp("hid", [BL, H], F32R)
    inp("enc", [BL * S, H], F32R)
    inp("W1", [H, H], F32R)
    inp("b1", [1, H], F32R)
    inp("W2", [H, H], F32R)
    inp("b2t", [P, KT], F32)
    inp("Vw", [H, 1], F32R)
    inp("emb", [VOCAB, EMB], F32)
    for l in range(1, 5):
        inp(f"Wk{l}", [2 * H, 3 * HL], F32R)
        inp(f"bl{l}", [1, 3 * HL], F32R)
    inp("fcW", [H, VL], F32R)
    inp("fcb", [1, VL], F32R)
    inp("ones", [1, B], F32R)

    o_logits = nc.dram_tensor("logits", [B, VL], F32, kind="ExternalOutput").ap()
    o_h4 = nc.dram_tensor("h4", [B, HL], F32, kind="ExternalOutput").ap()
    o_attw = nc.dram_tensor("attw", [BL, S], F32, kind="ExternalOutput").ap()

    with tile.TileContext(nc) as tc:
        with (
            tc.tile_pool(name="const", bufs=1) as const,
            tc.tile_pool(name="big", bufs=1) as big,
            tc.tile_pool(name="stream", bufs=2) as stream,
            tc.tile_pool(name="small", bufs=1) as small,
            tc.tile_pool(name="dram", bufs=1, space="DRAM") as dram,
        ):
            ident = const.tile([P, P], F32)
            make_identity(nc, ident)
            ones = const.tile([1, B], F32R)
            nc.sync.dma_start(ones[:], d["ones"][:])

            # fcW k-tiles: streamed through FCW_BUFS resident slots (gpsimd
            # queue so the prefetch never blocks latency-critical sync DMAs)
            fcw_tiles = []
            for k in range(KT):
                t = big.tile([P, VL], F32R, name=f"fcw{k}", tag="fcw",
                             bufs=FCW_BUFS)
                nc.gpsimd.dma_start(t[:], d["fcW"][k * P : (k + 1) * P, :])
                fcw_tiles.append(t)

            # ---- embedding gather ----
            idx = small.tile([B, 1], I32)
            nc.sync.dma_start(idx[:], d["x_idx"][:])

            att_big_cm = tc.tile_pool(name="att_big", bufs=1)
            att_big = att_big_cm.__enter__()
            with tc.tile_pool(name="ps_a", bufs=2, space="PSUM") as ps_a:
                ex = att_big.tile([B, EMB], F32, tag="ex")
                nc.gpsimd.indirect_dma_start(
                    out=ex[:],
                    out_offset=None,
                    in_=d["emb"][:],
                    in_offset=bass.IndirectOffsetOnAxis(ap=idx[:, :1], axis=0),
                )
                exT = []
                for k in range(KT):
                    pt = ps_a.tile([P, B], F32, space="PSUM", tag="trB")
                    nc.tensor.transpose(
                        pt[:], ex[:, k * P : (k + 1) * P], ident[:B, :B]
                    )
                    t = small.tile([P, B], F32R, name=f"exT{k}", tag=f"exT{k}")
                    nc.vector.tensor_copy(t[:], pt[:])
                    exT.append(t)

                # ---- hidden slices -> hidT k-tiles [128, 8] ----
                hidT = []
                for k in range(KT):
                    hsl = stream.tile([BL, P], F32R, tag="hsl", bufs=2)
                    nc.sync.dma_start(hsl[:], d["hid"][:, k * P : (k + 1) * P])
                    pt = ps_a.tile([P, BL], F32, space="PSUM", tag="tr8")
                    nc.tensor.transpose(
                        pt[:], hsl[:].bitcast(F32), ident[:BL, :BL]
                    )
                    t = small.tile([P, BL], F32R, name=f"hidT{k}",
                                   tag=f"hidT{k}")
                    nc.vector.tensor_copy(t[:], pt[:])
                    hidT.append(t)

                # ---- enc load + transpose -> encT k-tiles [128, 1024] ----
                enc_nat = []
                for b in range(BL):
                    t = att_big.tile([S, H], F32R, name=f"enc{b}", tag=f"enc{b}")
                    nc.sync.dma_start(t[:], d["enc"][b * S : (b + 1) * S, :])
                    enc_nat.append(t)
                encT = []
                for k in range(KT):
                    t = att_big.tile([P, BL * S], F32R, name=f"encT{k}",
                                     tag=f"encT{k}")
                    encT.append(t)
                for b in range(BL):
                    for k in range(KT):
                        pt = ps_a.tile([P, P], F32, space="PSUM", tag="trP")
                        nc.tensor.transpose(
                            pt[:],
                            enc_nat[b][:, k * P : (k + 1) * P].bitcast(F32),
                            ident[:],
                        )
                        nc.vector.tensor_copy(
                            encT[k][:, b * S : (b + 1) * S], pt[:]
                        )

                # ---- w1h = hidden @ W1 + b1 (batch-major [8, 1024]) ----
                b1s = small.tile([1, H], F32R, tag="b1s")
                nc.sync.dma_start(b1s[:], d["b1"][:])
                ph_lo = ps_a.tile([BL, 512], F32, space="PSUM", tag="w1h_lo",
                                  bufs=1)
                ph_hi = ps_a.tile([BL, 512], F32, space="PSUM", tag="w1h_hi",
                                  bufs=1)
                for k in range(KT):
                    w1k = stream.tile([P, H], F32R, tag="w1k", bufs=2)
                    nc.sync.dma_start(w1k[:], d["W1"][k * P : (k + 1) * P, :])
                    nc.tensor.matmul(ph_lo[:], hidT[k][:], w1k[:, :512],
                                     start=(k == 0), stop=False)
                    nc.tensor.matmul(ph_hi[:], hidT[k][:], w1k[:, 512:],
                                     start=(k == 0), stop=False)
                nc.tensor.matmul(ph_lo[:], ones[:, :BL], b1s[:, :512],
                                 start=False, stop=True)
                nc.tensor.matmul(ph_hi[:], ones[:, :BL], b1s[:, 512:],
                                 start=False, stop=True)
                w1h = small.tile([BL, H], F32, tag="w1h")
                nc.vector.tensor_copy(w1h[:, :512], ph_lo[:])
                nc.vector.tensor_copy(w1h[:, 512:], ph_hi[:])

                # ---- w1hT m-tiles [128, 8] (+ b2 per-feature) ----
                b2s = small.tile([P, KT], F32, tag="b2s")
                nc.sync.dma_start(b2s[:], d["b2t"][:])
                w1hT = []
                for m in range(KT):
                    pt = ps_a.tile([P, BL], F32, space="PSUM", tag="tr8")
                    nc.tensor.transpose(
                        pt[:], w1h[:, m * P : (m + 1) * P], ident[:BL, :BL]
                    )
                    t = small.tile([P, BL], F32, name=f"w1hT{m}",
                                   tag=f"w1hT{m}")
                    nc.vector.tensor_tensor(
                        out=t[:],
                        in0=pt[:],
                        in1=b2s[:, m : m + 1].to_broadcast((P, BL)),
                        op=ALU.add,
                    )
                    w1hT.append(t)

            # ---- w2e (feature-major) + tanh + score ----
            with tc.tile_pool(name="ps_b", bufs=2, space="PSUM") as ps_b:
                vw = small.tile([P, KT], F32R, tag="vw")
                nc.sync.dma_start(
                    vw[:], d["Vw"][:, 0:1].rearrange("(m p) o -> (p o) m", p=P)
                )
                ps_sc_lo = ps_b.tile([1, 512], F32, space="PSUM", tag="sc_lo",
                                     bufs=1)
                ps_sc_hi = ps_b.tile([1, 512], F32, space="PSUM", tag="sc_hi",
                                     bufs=1)
                with tc.tile_pool(name="tm_pool", bufs=2) as tm_pool:
                    for m in range(KT):
                        p_lo = ps_b.tile([P, 512], F32, space="PSUM",
                                         tag="w2e_lo")
                        p_hi = ps_b.tile([P, 512], F32, space="PSUM",
                                         tag="w2e_hi")
                        for k in range(KT):
                            w2k = stream.tile([P, P], F32R, tag="w2k", bufs=6)
                            nc.sync.dma_start(
                                w2k[:],
                                d["W2"][k * P : (k + 1) * P,
                                        m * P : (m + 1) * P],
                            )
                            nc.tensor.matmul(
                                p_lo[:], w2k[:], encT[k][:, :512],
                                start=(k == 0), stop=(k == KT - 1),
                            )
                            nc.tensor.matmul(
                                p_hi[:], w2k[:], encT[k][:, 512:],
                                start=(k == 0), stop=(k == KT - 1),
                            )
                        tma = tm_pool.tile([P, BL * S], F32, tag="tma")
                        tm = tm_pool.tile([P, BL * S], F32R, tag="tm")
                        bc = w1hT[m][:].rearrange(
                            "p (b o) -> p b o", o=1
                        ).to_broadcast((P, BL, S))
                        nc.vector.tensor_tensor(
                            out=tma[:, :512].rearrange(
                                "p (b s) -> p b s", s=S),
                            in0=p_lo[:].rearrange("p (b s) -> p b s", s=S),
                            in1=bc[:, :BL // 2, :],
                            op=ALU.add,
                        )
                        nc.vector.tensor_tensor(
                            out=tma[:, 512:].rearrange(
                                "p (b s) -> p b s", s=S),
                            in0=p_hi[:].rearrange("p (b s) -> p b s", s=S),
                            in1=bc[:, BL // 2 :, :],
                            op=ALU.add,
                        )
                        nc.scalar.activation(tm[:], tma[:], AF.Tanh)
                        nc.tensor.matmul(
                            ps_sc_lo[:], vw[:, m : m + 1], tm[:, :512],
                            start=(m == 0), stop=(m == KT - 1),
                        )
                        nc.tensor.matmul(
                            ps_sc_hi[:], vw[:, m : m + 1], tm[:, 512:],
                            start=(m == 0), stop=(m == KT - 1),
                        )

                # ---- softmax over S (per batch) ----
                sc_row = small.tile([1, BL * S], F32, tag="sc_row")
                nc.vector.tensor_copy(sc_row[:, :512], ps_sc_lo[:])
                nc.vector.tensor_copy(sc_row[:, 512:], ps_sc_hi[:])

            sc_dram = dram.tile([1, BL * S], F32, tag="sc_dram")
            nc.sync.dma_start(sc_dram[:], sc_row[:])
            s8 = small.tile([BL, S], F32, tag="s8")
            nc.sync.dma_start(
                s8[:], sc_dram[0:1, :].rearrange("o (b s) -> (o b) s", b=BL)
            )
            mx = small.tile([BL, 1], F32, tag="mx")
            nc.vector.reduce_max(mx[:], s8[:], axis=AX.X)
            nmx = small.tile([BL, 1], F32, tag="nmx")
            nc.vector.tensor_scalar_mul(nmx[:], mx[:], -1.0)
            e8 = small.tile([BL, S], F32, tag="e8")
            ssum = small.tile([BL, 1], F32, tag="ssum")
            nc.scalar.activation(e8[:], s8[:], AF.Exp, bias=nmx[:],
                                 accum_out=ssum[:])
            rsum = small.tile([BL, 1], F32, tag="rsum")
            nc.vector.reciprocal(rsum[:], ssum[:])
            w8 = small.tile([BL, S], F32, tag="w8")
            nc.vector.tensor_scalar_mul(w8[:], e8[:], rsum[:])
            nc.sync.dma_start(o_attw[:], w8[:])

            # ---- context, feature-major ----
            cc_ctx_in = dram.tile([H, BL], F32R, tag="cc_ctx_in")
            cc_ctx_out = dram.tile([NCORES, H, BL], F32R, tag="cc_ctx_out")
            with tc.tile_pool(name="ps_c", bufs=2, space="PSUM") as ps_c:
                ptw = ps_c.tile([S, BL], F32, space="PSUM", tag="trw", bufs=1)
                nc.tensor.transpose(ptw[:], w8[:], ident[:BL, :BL])
                wT = small.tile([S, BL], F32, tag="wT")
                nc.vector.tensor_copy(wT[:], ptw[:])
                for f in range(KT):
                    pc = ps_c.tile([P, BL], F32, space="PSUM", tag="ctx")
                    for b in range(BL):
                        nc.tensor.matmul(
                            pc[:, b : b + 1],
                            enc_nat[b][:, f * P : (f + 1) * P].bitcast(F32),
                            wT[:, b : b + 1],
                            start=True, stop=True,
                        )
                    ct = small.tile([P, BL], F32R, tag="ctxT", bufs=2)
                    nc.vector.tensor_copy(ct[:], pc[:])
                    nc.sync.dma_start(cc_ctx_in[f * P : (f + 1) * P, :], ct[:])
            att_big_cm.__exit__(None, None, None)
            nc.gpsimd.collective_compute(
                "AllGather", ALU.bypass, replica_groups=RG,
                ins=[cc_ctx_in[:].opt()], outs=[cc_ctx_out[:].opt()],
            )
            xt_ctx = []
            for k in range(KT):
                t = small.tile([P, B], F32R, name=f"xtctx{k}", tag=f"xtctx{k}")
                nc.sync.dma_start(
                    t[:].rearrange("p (c j) -> p c j", c=NCORES),
                    cc_ctx_out[:, k * P : (k + 1) * P, :].transpose([1, 0, 2]),
                )
                xt_ctx.append(t)

            # ---- LSTM stack (gate-column sharded) ----
            prevT = exT
            with tc.tile_pool(name="ps_d", bufs=2, space="PSUM") as ps_d:
                for l in range(1, 5):
                    bls = small.tile([1, 3 * HL], F32R, tag="bls", bufs=2)
                    nc.sync.dma_start(bls[:], d[f"bl{l}"][:])
                    pg = ps_d.tile([B, 3 * HL], F32, space="PSUM", tag="lstm")
                    xt_all = xt_ctx + prevT
                    for k in range(2 * KT):
                        wkt = stream.tile([P, 3 * HL], F32R, tag="wkt", bufs=6)
                        nc.sync.dma_start(
                            wkt[:], d[f"Wk{l}"][k * P : (k + 1) * P, :]
                        )
                        nc.tensor.matmul(pg[:], xt_all[k][:], wkt[:],
                                         start=(k == 0), stop=False)
                    nc.tensor.matmul(pg[:], ones[:], bls[:],
                                     start=False, stop=True)
                    # gates: i=[0:HL] g=[HL:2HL] o=[2HL:3HL]
                    ci = small.tile([B, HL], F32, tag="ci")
                    nc.scalar.activation(ci[:], pg[:, :HL], AF.Sigmoid)
                    tg = small.tile([B, HL], F32, tag="tg")
                    nc.scalar.activation(tg[:], pg[:, HL : 2 * HL], AF.Tanh)
                    cst = small.tile([B, HL], F32, tag="cst")
                    nc.vector.tensor_tensor(out=cst[:], in0=ci[:], in1=tg[:],
                                            op=ALU.mult)
                    tc2 = small.tile([B, HL], F32, tag="tc2")
                    nc.scalar.activation(tc2[:], cst[:], AF.Tanh)
                    so = small.tile([B, HL], F32, tag="so")
                    nc.scalar.activation(so[:], pg[:, 2 * HL :], AF.Sigmoid)
                    hsb = small.tile([B, HL], F32, tag="hsb")
                    nc.vector.tensor_tensor(out=hsb[:], in0=so[:], in1=tc2[:],
                                            op=ALU.mult)
                    if l == 4:
                        nc.sync.dma_start(o_h4[:], hsb[:])
                    pt = ps_d.tile([HL, B], F32, space="PSUM", tag="trB")
                    nc.tensor.transpose(pt[:], hsb[:], ident[:B, :B])
                    hT_chunk = small.tile([HL, B], F32R, tag="hT_chunk",
                                          bufs=2)
                    nc.vector.tensor_copy(hT_chunk[:], pt[:])
                    cc_h_in = dram.tile([HL, B], F32R, tag=f"cc_h_in{l}")
                    cc_h_out = dram.tile([H, B], F32R, tag=f"cc_h_out{l}")
                    nc.sync.dma_start(cc_h_in[:], hT_chunk[:])
                    nc.gpsimd.collective_compute(
                        "AllGather", ALU.bypass, replica_groups=RG,
                        ins=[cc_h_in[:].opt()], outs=[cc_h_out[:].opt()],
                    )
                    newT = []
                    for k in range(KT):
                        t = small.tile([P, B], F32R, name=f"hT{l}_{k}",
                                       tag=f"hT{l % 2}_{k}")
                        nc.sync.dma_start(
                            t[:], cc_h_out[k * P : (k + 1) * P, :]
                        )
                        newT.append(t)
                    prevT = newT

            # ---- fc: logits = h4 @ fcW + fcb (vocab-sharded, k-outer) ----
            with tc.tile_pool(name="ps_e", bufs=1, space="PSUM") as ps_e:
                pf = [
                    ps_e.tile([B, NW], F32, space="PSUM", tag=f"fc{n}",
                              name=f"fc{n}")
                    for n in range(NT)
                ]
                for k in range(KT):
                    for n in range(NT):
                        nc.tensor.matmul(
                            pf[n][:],
                            prevT[k][:],
                            fcw_tiles[k][:, n * NW : (n + 1) * NW],
                            start=(k == 0), stop=False,
                        )
                for n in range(NT):
                    fcbn = stream.tile([1, NW], F32R, tag="fcbn", bufs=2)
                    nc.sync.dma_start(
                        fcbn[:], d["fcb"][0:1, n * NW : (n + 1) * NW]
                    )
                    nc.tensor.matmul(pf[n][:], ones[:], fcbn[:],
                                     start=False, stop=True)
                    lg = small.tile([B, NW], F32, tag="lg", bufs=2)
                    nc.vector.tensor_copy(lg[:], pf[n][:])
                    nc.sync.dma_start(
                        o_logits[:, n * NW : (n + 1) * NW], lg[:]
                    )

    nc.finalize()
    return nc


_NC = None


def _get_nc():
    global _NC
    if _NC is None:
        _NC = _build()
    return _NC


def _prep_in_maps(inputs):
    return _shard(**{k: np.asarray(v) for k, v in inputs.items()})


def _shard(x, hidden, enc_output, W1, b1, W2, b2, Vw, Vb, emb,
           Wk1, Wr1, bl1, Wk2, Wr2, bl2, Wk3, Wr3, bl3, Wk4, Wr4, bl4,
           fcW, fcb):
    f32 = np.float32
    x = np.ascontiguousarray(np.asarray(x).astype(np.int32).reshape(B, 1))
    hidden = np.asarray(hidden, f32)
    enc = np.asarray(enc_output, f32).reshape(B * S, H)
    W1 = np.ascontiguousarray(np.asarray(W1, f32))
    W2 = np.ascontiguousarray(np.asarray(W2, f32))
    b1 = np.asarray(b1, f32).reshape(1, H)
    b2t = np.ascontiguousarray(np.asarray(b2, f32).reshape(KT, P).T)
    Vw = np.ascontiguousarray(np.asarray(Vw, f32).reshape(H, 1))
    emb = np.ascontiguousarray(np.asarray(emb, f32))
    fcW = np.ascontiguousarray(np.asarray(fcW, f32))
    fcb = np.asarray(fcb, f32).reshape(1, VOCAB)
    ones = np.ones((1, B), f32)
    Wks = [np.asarray(w, f32) for w in (Wk1, Wk2, Wk3, Wk4)]
    bls = [np.asarray(v, f32).reshape(4 * H) for v in (bl1, bl2, bl3, bl4)]

    in_maps = []
    for c in range(NCORES):
        bsl = slice(c * BL, (c + 1) * BL)
        csl = [slice(g * H + c * HL, g * H + (c + 1) * HL) for g in (0, 2, 3)]
        m = {
            "x_idx": x,
            "hid": np.ascontiguousarray(hidden[bsl]),
            "enc": np.ascontiguousarray(enc[c * BL * S : (c + 1) * BL * S]),
            "W1": W1, "b1": b1, "W2": W2, "b2t": b2t, "Vw": Vw, "emb": emb,
            "fcW": np.ascontiguousarray(fcW[:, c * VL : (c + 1) * VL]),
            "fcb": np.ascontiguousarray(fcb[:, c * VL : (c + 1) * VL]),
            "ones": ones,
        }
        for l in range(4):
            m[f"Wk{l + 1}"] = np.ascontiguousarray(
                np.concatenate([Wks[l][:, s] for s in csl], axis=1)
            )
            m[f"bl{l + 1}"] = np.ascontiguousarray(
                np.concatenate([bls[l][s] for s in csl]).reshape(1, 3 * HL)
            )
        in_maps.append(m)
    return in_maps


def kernel(**inputs):
    nc = _get_nc()
    in_maps = _prep_in_maps(inputs)
    res = run_bass_kernel_spmd(nc, in_maps, core_ids=list(range(NCORES)))
    outs = res.results
    logits = np.concatenate([outs[c]["logits"] for c in range(NCORES)], axis=1)
    h4 = np.concatenate([outs[c]["h4"] for c in range(NCORES)], axis=1)
    attw = np.concatenate([outs[c]["attw"] for c in range(NCORES)], axis=0)
    return logits, h4, attw.reshape(B, S, 1)


# revision 14
# speedup vs baseline: 1.0240x; 1.0240x over previous
"""Trainium2 Bass kernel for nn_Decoder: Bahdanau attention + 4-layer LSTM
(single step, zero initial state) + vocab projection, on 8 NeuronCores.

Sharding: attention is data-parallel over batch (8 batches/core); the LSTM
stack is tensor-parallel over gate columns (128 of 1024 h-features per core,
only i/g/o gates — the f gate multiplies zero state and Wr multiplies h0=0);
the fc vocab projection is tensor-parallel over vocab (4000 cols/core).
Feature-major (transposed) activations flow through the LSTM so every matmul
has its contraction dim on partitions; 5 small AllGathers (context + 4 h's)
stitch the cores together.  Big matmuls run in float32r (~1e-4 rel err).
Attention inputs arrive pre-transposed from the host; dummy "warm" matmuls
keep the PE HAM clock-gate at 2.4 GHz across DMA/collective gaps.
"""
import numpy as np

import concourse.bass as bass
import concourse.mybir as mybir
import concourse.tile as tile
from concourse import bacc
from concourse.bass_utils import run_bass_kernel_spmd
from concourse.masks import make_identity

P = 128
NCORES = 8
B, S, H, EMB, VOCAB = 64, 128, 1024, 1024, 32000
BL = B // NCORES          # 8 batches per core
HL = H // NCORES          # 128 gate columns / h features per core
VL = VOCAB // NCORES      # 4000 vocab columns per core
KT = H // P               # 8 k-tiles over a 1024 feature dim
NT = 8                    # fc n-tiles
NW = VL // NT             # 500

F32 = mybir.dt.float32
F32R = mybir.dt.float32r
I32 = mybir.dt.int32
AF = mybir.ActivationFunctionType
ALU = mybir.AluOpType
AX = mybir.AxisListType

FCW_BUFS = 5   # fcW k-tile slots resident (of 8 tiles, 2 MB each)
RG = [list(range(NCORES))]


def _build():
    nc = bacc.Bacc(
        "TRN2", target_bir_lowering=False, debug=False, num_devices=NCORES
    )

    d = {}

    def inp(name, shape, dt):
        d[name] = nc.dram_tensor(name, shape, dt, kind="ExternalInput").ap()

    inp("x_idx", [B, 1], I32)
    inp("hidT", [H, BL], F32R)       # hidden slice, pre-transposed
    inp("encT", [H, BL * S], F32R)   # enc slice, pre-transposed (f, (b,s))
    inp("W1", [H, H], F32R)
    inp("b1", [1, H], F32R)
    inp("W2", [H, H], F32R)
    inp("b2t", [P, KT], F32)
    inp("Vw", [H, 1], F32R)
    inp("emb", [VOCAB, EMB], F32)
    for l in range(1, 5):
        inp(f"Wk{l}", [2 * H, 3 * HL], F32R)
        inp(f"bl{l}", [1, 3 * HL], F32R)
    inp("fcW", [H, VL], F32R)
    inp("fcb", [1, VL], F32R)
    inp("ones", [1, B], F32R)

    o_logits = nc.dram_tensor("logits", [B, VL], F32, kind="ExternalOutput").ap()
    o_h4 = nc.dram_tensor("h4", [B, HL], F32, kind="ExternalOutput").ap()
    o_attw = nc.dram_tensor("attw", [BL, S], F32, kind="ExternalOutput").ap()

    with tile.TileContext(nc) as tc:
        with (
            tc.tile_pool(name="const", bufs=1) as const,
            tc.tile_pool(name="big", bufs=1) as big,
            tc.tile_pool(name="stream", bufs=2) as stream,
            tc.tile_pool(name="small", bufs=1) as small,
            tc.tile_pool(name="dram", bufs=1, space="DRAM") as dram,
        ):
            ident = const.tile([P, P], F32)
            make_identity(nc, ident)
            ones = const.tile([1, B], F32R)
            nc.sync.dma_start(ones[:], d["ones"][:])
            # junk operands for HAM warm-keeper matmuls (f32 = slow = good)
            wl = const.tile([P, P], F32)
            nc.any.memset(wl[:], 0.0)
            wr = const.tile([P, 512], F32)
            nc.any.memset(wr[:], 0.0)

            def warm(ps_pool, n):
                for _ in range(n):
                    pw = ps_pool.tile([P, 512], F32, space="PSUM", tag="warm",
                                      bufs=1, name="pw")
                    nc.tensor.matmul(pw[:], wl[:], wr[:], start=True,
                                     stop=True)

            # fcW k-tiles stream through FCW_BUFS slots on the gpsimd queue
            fcw_tiles = []
            for k in range(KT):
                t = big.tile([P, VL], F32R, name=f"fcw{k}", tag="fcw",
                             bufs=FCW_BUFS)
                nc.gpsimd.dma_start(t[:], d["fcW"][k * P : (k + 1) * P, :])
                fcw_tiles.append(t)

            # warm up the collective path with a tiny AllGather
            wcc_in = dram.tile([BL, 8], F32, tag="wcc_in")
            wcc_out = dram.tile([B, 8], F32, tag="wcc_out")
            zz = small.tile([BL, 8], F32, tag="zz")
            nc.any.memset(zz[:], 0.0)
            nc.sync.dma_start(wcc_in[:], zz[:])
            nc.gpsimd.collective_compute(
                "AllGather", ALU.bypass, replica_groups=RG,
                ins=[wcc_in[:].opt()], outs=[wcc_out[:].opt()],
            )

            # ---- embedding gather ----
            idx = small.tile([B, 1], I32)
            nc.sync.dma_start(idx[:], d["x_idx"][:])

            att_big_cm = tc.tile_pool(name="att_big", bufs=1)
            att_big = att_big_cm.__enter__()
            with tc.tile_pool(name="ps_a", bufs=2, space="PSUM") as ps_a:
                warm(ps_a, 8)

                # encT k-tiles straight from DRAM
                encT = []
                for k in range(KT):
                    t = att_big.tile([P, BL * S], F32R, name=f"encT{k}",
                                     tag=f"encT{k}")
                    nc.sync.dma_start(t[:], d["encT"][k * P : (k + 1) * P, :])
                    encT.append(t)

                # hidT k-tiles straight from DRAM
                hidT = []
                for k in range(KT):
                    t = small.tile([P, BL], F32R, name=f"hidT{k}",
                                   tag=f"hidT{k}")
                    nc.sync.dma_start(t[:], d["hidT"][k * P : (k + 1) * P, :])
                    hidT.append(t)

                # embedding rows -> exT k-tiles [128, 64] via PE transpose
                ex = att_big.tile([B, EMB], F32, tag="ex")
                nc.gpsimd.indirect_dma_start(
                    out=ex[:],
                    out_offset=None,
                    in_=d["emb"][:],
                    in_offset=bass.IndirectOffsetOnAxis(ap=idx[:, :1], axis=0),
                )
                exT = []
                for k in range(KT):
                    pt = ps_a.tile([P, B], F32, space="PSUM", tag="trB")
                    nc.tensor.transpose(
                        pt[:], ex[:, k * P : (k + 1) * P], ident[:B, :B]
                    )
                    t = small.tile([P, B], F32R, name=f"exT{k}", tag=f"exT{k}")
                    nc.vector.tensor_copy(t[:], pt[:])
                    exT.append(t)

                # ---- w1h = hidden @ W1 + b1 (batch-major [8, 1024]) ----
                b1s = small.tile([1, H], F32R, tag="b1s")
                nc.sync.dma_start(b1s[:], d["b1"][:])
                ph_lo = ps_a.tile([BL, 512], F32, space="PSUM", tag="w1h_lo",
                                  bufs=1)
                ph_hi = ps_a.tile([BL, 512], F32, space="PSUM", tag="w1h_hi",
                                  bufs=1)
                for k in range(KT):
                    w1k = stream.tile([P, H], F32R, tag="w1k", bufs=2)
                    nc.sync.dma_start(w1k[:], d["W1"][k * P : (k + 1) * P, :])
                    nc.tensor.matmul(ph_lo[:], hidT[k][:], w1k[:, :512],
                                     start=(k == 0), stop=False)
                    nc.tensor.matmul(ph_hi[:], hidT[k][:], w1k[:, 512:],
                                     start=(k == 0), stop=False)
                nc.tensor.matmul(ph_lo[:], ones[:, :BL], b1s[:, :512],
                                 start=False, stop=True)
                nc.tensor.matmul(ph_hi[:], ones[:, :BL], b1s[:, 512:],
                                 start=False, stop=True)
                w1h = small.tile([BL, H], F32, tag="w1h")
                nc.vector.tensor_copy(w1h[:, :512], ph_lo[:])
                nc.vector.tensor_copy(w1h[:, 512:], ph_hi[:])

                # ---- w1hT m-tiles [128, 8] (+ b2 per-feature) ----
                b2s = small.tile([P, KT], F32, tag="b2s")
                nc.sync.dma_start(b2s[:], d["b2t"][:])
                w1hT = []
                for m in range(KT):
                    pt = ps_a.tile([P, BL], F32, space="PSUM", tag="tr8")
                    nc.tensor.transpose(
                        pt[:], w1h[:, m * P : (m + 1) * P], ident[:BL, :BL]
                    )
                    t = small.tile([P, BL], F32, name=f"w1hT{m}",
                                   tag=f"w1hT{m}")
                    nc.vector.tensor_tensor(
                        out=t[:],
                        in0=pt[:],
                        in1=b2s[:, m : m + 1].to_broadcast((P, BL)),
                        op=ALU.add,
                    )
                    w1hT.append(t)

            # ---- w2e (feature-major) + tanh + score ----
            with tc.tile_pool(name="ps_b", bufs=2, space="PSUM") as ps_b:
                vw = small.tile([P, KT], F32R, tag="vw")
                nc.sync.dma_start(
                    vw[:], d["Vw"][:, 0:1].rearrange("(m p) o -> (p o) m", p=P)
                )
                ps_sc_lo = ps_b.tile([1, 512], F32, space="PSUM", tag="sc_lo",
                                     bufs=1)
                ps_sc_hi = ps_b.tile([1, 512], F32, space="PSUM", tag="sc_hi",
                                     bufs=1)
                with tc.tile_pool(name="tm_pool", bufs=2) as tm_pool:
                    for m in range(KT):
                        # one m-block of W2 as a [128, (kt, m')] tile
                        w2m = tm_pool.tile([P, H], F32R, tag="w2m", bufs=2)
                        nc.sync.dma_start(
                            w2m[:].rearrange("p (kt mm) -> p kt mm", kt=KT),
                            d["W2"][:, m * P : (m + 1) * P].rearrange(
                                "(kt kp) mm -> kp kt mm", kp=P
                            ),
                        )
                        p_lo = ps_b.tile([P, 512], F32, space="PSUM",
                                         tag="w2e_lo")
                        p_hi = ps_b.tile([P, 512], F32, space="PSUM",
                                         tag="w2e_hi")
                        for k in range(KT):
                            lhs = w2m[:, k * P : (k + 1) * P]
                            nc.tensor.matmul(
                                p_lo[:], lhs, encT[k][:, :512],
                                start=(k == 0), stop=(k == KT - 1),
                            )
                            nc.tensor.matmul(
                                p_hi[:], lhs, encT[k][:, 512:],
                                start=(k == 0), stop=(k == KT - 1),
                            )
                        tma = tm_pool.tile([P, BL * S], F32, tag="tma")
                        tm = tm_pool.tile([P, BL * S], F32R, tag="tm")
                        bc = w1hT[m][:].rearrange(
                            "p (b o) -> p b o", o=1
                        ).to_broadcast((P, BL, S))
                        nc.vector.tensor_tensor(
                            out=tma[:, :512].rearrange(
                                "p (b s) -> p b s", s=S),
                            in0=p_lo[:].rearrange("p (b s) -> p b s", s=S),
                            in1=bc[:, :BL // 2, :],
                            op=ALU.add,
                        )
                        nc.vector.tensor_tensor(
                            out=tma[:, 512:].rearrange(
                                "p (b s) -> p b s", s=S),
                            in0=p_hi[:].rearrange("p (b s) -> p b s", s=S),
                            in1=bc[:, BL // 2 :, :],
                            op=ALU.add,
                        )
                        nc.scalar.activation(tm[:], tma[:], AF.Tanh)
                        nc.tensor.matmul(
                            ps_sc_lo[:], vw[:, m : m + 1], tm[:, :512],
                            start=(m == 0), stop=(m == KT - 1),
                        )
                        nc.tensor.matmul(
                            ps_sc_hi[:], vw[:, m : m + 1], tm[:, 512:],
                            start=(m == 0), stop=(m == KT - 1),
                        )

                # ---- softmax over S (per batch) ----
                sc_row = small.tile([1, BL * S], F32, tag="sc_row")
                nc.vector.tensor_copy(sc_row[:, :512], ps_sc_lo[:])
                nc.vector.tensor_copy(sc_row[:, 512:], ps_sc_hi[:])

            sc_dram = dram.tile([1, BL * S], F32, tag="sc_dram")
            nc.sync.dma_start(sc_dram[:], sc_row[:])
            s8 = small.tile([BL, S], F32, tag="s8")
            nc.sync.dma_start(
                s8[:], sc_dram[0:1, :].rearrange("o (b s) -> (o b) s", b=BL)
            )
            mx = small.tile([BL, 1], F32, tag="mx")
            nc.vector.reduce_max(mx[:], s8[:], axis=AX.X)
            nmx = small.tile([BL, 1], F32, tag="nmx")
            nc.vector.tensor_scalar_mul(nmx[:], mx[:], -1.0)
            e8 = small.tile([BL, S], F32, tag="e8")
            ssum = small.tile([BL, 1], F32, tag="ssum")
            nc.scalar.activation(e8[:], s8[:], AF.Exp, bias=nmx[:],
                                 accum_out=ssum[:])
            rsum = small.tile([BL, 1], F32, tag="rsum")
            nc.vector.reciprocal(rsum[:], ssum[:])
            w8 = small.tile([BL, S], F32, tag="w8")
            nc.vector.tensor_scalar_mul(w8[:], e8[:], rsum[:])
            nc.sync.dma_start(o_attw[:], w8[:])

            # ---- context via DVE on encT: ctxT[f, b] = sum_s encT*w ----
            w_dram = dram.tile([1, BL * S], F32, tag="w_dram")
            nc.sync.dma_start(
                w_dram[0:1, :].rearrange("o (b s) -> (o b) s", b=BL), w8[:]
            )
            w128 = big.tile([P, BL * S], F32, tag="w128")
            nc.sync.dma_start(
                w128[:], w_dram[0:1, :].to_broadcast((P, BL * S))
            )
            cc_ctx_in = dram.tile([H, BL], F32R, tag="cc_ctx_in")
            cc_ctx_out = dram.tile([NCORES, H, BL], F32R, tag="cc_ctx_out")
            for f in range(KT):
                prod = big.tile([P, BL * S], F32, tag="prod", bufs=1)
                nc.vector.tensor_tensor(
                    out=prod[:], in0=encT[f][:].bitcast(F32), in1=w128[:],
                    op=ALU.mult,
                )
                ct = small.tile([P, BL], F32R, tag="ctxT", bufs=2)
                with nc.allow_low_precision(reason="f32r is full-width"):
                    nc.vector.reduce_sum(
                        ct[:], prod[:].rearrange("p (b s) -> p b s", s=S),
                        axis=AX.X,
                    )
                nc.sync.dma_start(cc_ctx_in[f * P : (f + 1) * P, :], ct[:])
            att_big_cm.__exit__(None, None, None)
            nc.gpsimd.collective_compute(
                "AllGather", ALU.bypass, replica_groups=RG,
                ins=[cc_ctx_in[:].opt()], outs=[cc_ctx_out[:].opt()],
            )
            xt_ctx = []
            for k in range(KT):
                t = small.tile([P, B], F32R, name=f"xtctx{k}", tag=f"xtctx{k}")
                nc.sync.dma_start(
                    t[:].rearrange("p (c j) -> p c j", c=NCORES),
                    cc_ctx_out[:, k * P : (k + 1) * P, :].transpose([1, 0, 2]),
                )
                xt_ctx.append(t)

            # ---- LSTM stack (gate-column sharded) ----
            prevT = exT
            with tc.tile_pool(name="ps_d", bufs=2, space="PSUM") as ps_d:
                warm(ps_d, 12)
                for l in range(1, 5):
                    bls = small.tile([1, 3 * HL], F32R, tag="bls", bufs=2)
                    nc.sync.dma_start(bls[:], d[f"bl{l}"][:])
                    pg = ps_d.tile([B, 3 * HL], F32, space="PSUM", tag="lstm")
                    xt_all = xt_ctx + prevT
                    for k in range(2 * KT):
                        wkt = stream.tile([P, 3 * HL], F32R, tag="wkt", bufs=6)
                        nc.sync.dma_start(
                            wkt[:], d[f"Wk{l}"][k * P : (k + 1) * P, :]
                        )
                        nc.tensor.matmul(pg[:], xt_all[k][:], wkt[:],
                                         start=(k == 0), stop=False)
                    nc.tensor.matmul(pg[:], ones[:], bls[:],
                                     start=False, stop=True)
                    ci = small.tile([B, HL], F32, tag="ci")
                    nc.scalar.activation(ci[:], pg[:, :HL], AF.Sigmoid)
                    tg = small.tile([B, HL], F32, tag="tg")
                    nc.scalar.activation(tg[:], pg[:, HL : 2 * HL], AF.Tanh)
                    cst = small.tile([B, HL], F32, tag="cst")
                    nc.vector.tensor_tensor(out=cst[:], in0=ci[:], in1=tg[:],
                                            op=ALU.mult)
                    tc2 = small.tile([B, HL], F32, tag="tc2")
                    nc.scalar.activation(tc2[:], cst[:], AF.Tanh)
                    so = small.tile([B, HL], F32, tag="so")
                    nc.scalar.activation(so[:], pg[:, 2 * HL :], AF.Sigmoid)
                    hsb = small.tile([B, HL], F32, tag="hsb")
                    nc.vector.tensor_tensor(out=hsb[:], in0=so[:], in1=tc2[:],
                                            op=ALU.mult)
                    if l == 4:
                        nc.sync.dma_start(o_h4[:], hsb[:])
                    pt = ps_d.tile([HL, B], F32, space="PSUM", tag="trB")
                    nc.tensor.transpose(pt[:], hsb[:], ident[:B, :B])
                    hT_chunk = small.tile([HL, B], F32R, tag="hT_chunk",
                                          bufs=2)
                    nc.vector.tensor_copy(hT_chunk[:], pt[:])
                    cc_h_in = dram.tile([HL, B], F32R, tag=f"cc_h_in{l}")
                    cc_h_out = dram.tile([H, B], F32R, tag=f"cc_h_out{l}")
                    nc.sync.dma_start(cc_h_in[:], hT_chunk[:])
                    nc.gpsimd.collective_compute(
                        "AllGather", ALU.bypass, replica_groups=RG,
                        ins=[cc_h_in[:].opt()], outs=[cc_h_out[:].opt()],
                    )
                    warm(ps_d, 8)
                    newT = []
                    for k in range(KT):
                        t = small.tile([P, B], F32R, name=f"hT{l}_{k}",
                                       tag=f"hT{l % 2}_{k}")
                        nc.sync.dma_start(
                            t[:], cc_h_out[k * P : (k + 1) * P, :]
                        )
                        newT.append(t)
                    prevT = newT

            # ---- fc: logits = h4 @ fcW + fcb (vocab-sharded, k-outer) ----
            with tc.tile_pool(name="ps_e", bufs=1, space="PSUM") as ps_e:
                pf = [
                    ps_e.tile([B, NW], F32, space="PSUM", tag=f"fc{n}",
                              name=f"fc{n}")
                    for n in range(NT)
                ]
                for k in range(KT):
                    for n in range(NT):
                        nc.tensor.matmul(
                            pf[n][:],
                            prevT[k][:],
                            fcw_tiles[k][:, n * NW : (n + 1) * NW],
                            start=(k == 0), stop=False,
                        )
                for n in range(NT):
                    fcbn = stream.tile([1, NW], F32R, tag="fcbn", bufs=2)
                    nc.sync.dma_start(
                        fcbn[:], d["fcb"][0:1, n * NW : (n + 1) * NW]
                    )
                    nc.tensor.matmul(pf[n][:], ones[:], fcbn[:],
                                     start=False, stop=True)
                    lg = small.tile([B, NW], F32, tag="lg", bufs=2)
                    nc.vector.tensor_copy(lg[:], pf[n][:])
                    nc.sync.dma_start(
                        o_logits[:, n * NW : (n + 1) * NW], lg[:]
                    )

    nc.finalize()
    return nc


_NC = None


def _get_nc():
    global _NC
    if _NC is None:
        _NC = _build()
    return _NC


def _prep_in_maps(inputs):
    return _shard(**{k: np.asarray(v) for k, v in inputs.items()})


def _shard(x, hidden, enc_output, W1, b1, W2, b2, Vw, Vb, emb,
           Wk1, Wr1, bl1, Wk2, Wr2, bl2, Wk3, Wr3, bl3, Wk4, Wr4, bl4,
           fcW, fcb):
    f32 = np.float32
    x = np.ascontiguousarray(np.asarray(x).astype(np.int32).reshape(B, 1))
    hidden = np.asarray(hidden, f32)
    enc = np.asarray(enc_output, f32).reshape(B, S, H)
    W1 = np.ascontiguousarray(np.asarray(W1, f32))
    W2 = np.ascontiguousarray(np.asarray(W2, f32))
    b1 = np.asarray(b1, f32).reshape(1, H)
    b2t = np.ascontiguousarray(np.asarray(b2, f32).reshape(KT, P).T)
    Vw = np.ascontiguousarray(np.asarray(Vw, f32).reshape(H, 1))
    emb = np.ascontiguousarray(np.asarray(emb, f32))
    fcW = np.ascontiguousarray(np.asarray(fcW, f32))
    fcb = np.asarray(fcb, f32).reshape(1, VOCAB)
    ones = np.ones((1, B), f32)
    Wks = [np.asarray(w, f32) for w in (Wk1, Wk2, Wk3, Wk4)]
    bls = [np.asarray(v, f32).reshape(4 * H) for v in (bl1, bl2, bl3, bl4)]

    in_maps = []
    for c in range(NCORES):
        bsl = slice(c * BL, (c + 1) * BL)
        csl = [slice(g * H + c * HL, g * H + (c + 1) * HL) for g in (0, 2, 3)]
        m = {
            "x_idx": x,
            "hidT": np.ascontiguousarray(hidden[bsl].T),
            "encT": np.ascontiguousarray(enc[bsl].reshape(BL * S, H).T),
            "W1": W1, "b1": b1, "W2": W2, "b2t": b2t, "Vw": Vw, "emb": emb,
            "fcW": np.ascontiguousarray(fcW[:, c * VL : (c + 1) * VL]),
            "fcb": np.ascontiguousarray(fcb[:, c * VL : (c + 1) * VL]),
            "ones": ones,
        }
        for l in range(4):
            m[f"Wk{l + 1}"] = np.ascontiguousarray(
                np.concatenate([Wks[l][:, s] for s in csl], axis=1)
            )
            m[f"bl{l + 1}"] = np.ascontiguousarray(
                np.concatenate([bls[l][s] for s in csl]).reshape(1, 3 * HL)
            )
        in_maps.append(m)
    return in_maps


def kernel(**inputs):
    nc = _get_nc()
    in_maps = _prep_in_maps(inputs)
    res = run_bass_kernel_spmd(nc, in_maps, core_ids=list(range(NCORES)))
    outs = res.results
    logits = np.concatenate([outs[c]["logits"] for c in range(NCORES)], axis=1)
    h4 = np.concatenate([outs[c]["h4"] for c in range(NCORES)], axis=1)
    attw = np.concatenate([outs[c]["attw"] for c in range(NCORES)], axis=0)
    return logits, h4, attw.reshape(B, S, 1)


# revision 16
# speedup vs baseline: 1.1404x; 1.1138x over previous
"""Trainium2 Bass kernel for nn_Decoder: Bahdanau attention + 4-layer LSTM
(single step, zero initial state) + vocab projection, on 8 NeuronCores.

Sharding: attention is data-parallel over batch (8 batches/core); the LSTM
stack is tensor-parallel over gate columns (128 of 1024 h-features per core,
only i/g/o gates — the f gate multiplies zero state and Wr multiplies h0=0);
the fc vocab projection is tensor-parallel over vocab (4000 cols/core).
Feature-major (transposed) activations flow through the LSTM so every matmul
has its contraction dim on partitions; 5 small AllGathers (context + 4 h's)
stitch the cores together.  Big matmuls run in float32r (~1e-4 rel err).
Attention inputs arrive pre-transposed from the host; dummy "warm" matmuls
keep the PE HAM clock-gate at 2.4 GHz across DMA/collective gaps.
"""
import numpy as np

import concourse.bass as bass
import concourse.mybir as mybir
import concourse.tile as tile
from concourse import bacc
from concourse.bass_utils import run_bass_kernel_spmd
from concourse.masks import make_identity

P = 128
NCORES = 8
B, S, H, EMB, VOCAB = 64, 128, 1024, 1024, 32000
BL = B // NCORES          # 8 batches per core
HL = H // NCORES          # 128 gate columns / h features per core
VL = VOCAB // NCORES      # 4000 vocab columns per core
KT = H // P               # 8 k-tiles over a 1024 feature dim
NT = 8                    # fc n-tiles
NW = VL // NT             # 500

F32 = mybir.dt.float32
F32R = mybir.dt.float32r
I32 = mybir.dt.int32
AF = mybir.ActivationFunctionType
ALU = mybir.AluOpType
AX = mybir.AxisListType

FCW_BUFS = 5   # fcW k-tile slots resident (of 8 tiles, 2 MB each)
RG = [list(range(NCORES))]


def _build():
    nc = bacc.Bacc(
        "TRN2", target_bir_lowering=False, debug=False, num_devices=NCORES
    )

    d = {}

    def inp(name, shape, dt):
        d[name] = nc.dram_tensor(name, shape, dt, kind="ExternalInput").ap()

    inp("x_idx", [B, 1], I32)
    inp("hidT", [H, BL], F32R)       # hidden slice, pre-transposed
    inp("encT", [H, BL * S], F32R)   # enc slice, pre-transposed (f, (b,s))
    inp("W1", [H, H], F32R)
    inp("b1", [1, H], F32R)
    inp("W2", [H, H], F32R)
    inp("b2t", [P, KT], F32)
    inp("Vw", [H, 1], F32R)
    inp("emb", [VOCAB, EMB], F32)
    for l in range(1, 5):
        inp(f"Wk{l}", [2 * H, 3 * HL], F32R)
        inp(f"bl{l}", [1, 3 * HL], F32R)
    inp("fcW", [H, VL], F32R)
    inp("fcb", [1, VL], F32R)
    inp("ones", [1, B], F32R)

    o_logits = nc.dram_tensor("logits", [B, VL], F32, kind="ExternalOutput").ap()
    o_h4 = nc.dram_tensor("h4", [B, HL], F32, kind="ExternalOutput").ap()
    o_attw = nc.dram_tensor("attw", [BL, S], F32, kind="ExternalOutput").ap()

    with tile.TileContext(nc) as tc:
        with (
            tc.tile_pool(name="const", bufs=1) as const,
            tc.tile_pool(name="big", bufs=1) as big,
            tc.tile_pool(name="stream", bufs=2) as stream,
            tc.tile_pool(name="small", bufs=1) as small,
            tc.tile_pool(name="dram", bufs=1, space="DRAM") as dram,
        ):
            ident = const.tile([P, P], F32)
            make_identity(nc, ident)
            ones = const.tile([1, B], F32R)
            nc.sync.dma_start(ones[:], d["ones"][:])
            # junk operands for HAM warm-keeper matmuls (f32 = slow = good)
            wl = const.tile([P, P], F32)
            nc.any.memset(wl[:], 0.0)
            wr = const.tile([P, 512], F32)
            nc.any.memset(wr[:], 0.0)

            def warm(ps_pool, n):
                for _ in range(n):
                    pw = ps_pool.tile([P, 512], F32, space="PSUM", tag="warm",
                                      bufs=1, name="pw")
                    nc.tensor.matmul(pw[:], wl[:], wr[:], start=True,
                                     stop=True)

            # fcW k-tiles stream through FCW_BUFS slots; DMAs are emitted
            # in 0.5 MB chunks interleaved with critical-path DMAs so no
            # queue gets a 2 MB head-of-line blocker.
            fcw_tiles = [
                big.tile([P, VL], F32R, name=f"fcw{k}", tag="fcw",
                         bufs=FCW_BUFS)
                for k in range(KT)
            ]
            CH = VL // 4
            fcw_chunks = [(k, c) for k in range(KT) for c in range(4)]
            fcw_state = [0]

            def emit_fcw(n, upto=KT):
                while n > 0 and fcw_state[0] < len(fcw_chunks):
                    k, c = fcw_chunks[fcw_state[0]]
                    if k >= upto:
                        return
                    fcw_state[0] += 1
                    nc.sync.dma_start(
                        fcw_tiles[k][:, c * CH : (c + 1) * CH],
                        d["fcW"][k * P : (k + 1) * P, c * CH : (c + 1) * CH],
                    )
                    n -= 1

            # warm up the collective path with a tiny AllGather
            wcc_in = dram.tile([BL, 8], F32, tag="wcc_in")
            wcc_out = dram.tile([B, 8], F32, tag="wcc_out")
            zz = small.tile([BL, 8], F32, tag="zz")
            nc.any.memset(zz[:], 0.0)
            nc.sync.dma_start(wcc_in[:], zz[:])
            nc.gpsimd.collective_compute(
                "AllGather", ALU.bypass, replica_groups=RG,
                ins=[wcc_in[:].opt()], outs=[wcc_out[:].opt()],
            )
            wcc_out2 = dram.tile([B, 8], F32, tag="wcc_out2")
            nc.gpsimd.collective_compute(
                "AllGather", ALU.bypass, replica_groups=RG,
                ins=[wcc_in[:].opt()], outs=[wcc_out2[:].opt()],
            )

            # ---- embedding gather ----
            idx = small.tile([B, 1], I32)
            nc.sync.dma_start(idx[:], d["x_idx"][:])

            att_big_cm = tc.tile_pool(name="att_big", bufs=1)
            att_big = att_big_cm.__enter__()
            with tc.tile_pool(name="ps_a", bufs=2, space="PSUM") as ps_a:
                warm(ps_a, 8)

                # encT k-tiles straight from DRAM
                encT = []
                for k in range(KT):
                    t = att_big.tile([P, BL * S], F32R, name=f"encT{k}",
                                     tag=f"encT{k}")
                    nc.sync.dma_start(t[:], d["encT"][k * P : (k + 1) * P, :])
                    encT.append(t)

                # hidT k-tiles straight from DRAM
                hidT = []
                for k in range(KT):
                    t = small.tile([P, BL], F32R, name=f"hidT{k}",
                                   tag=f"hidT{k}")
                    nc.sync.dma_start(t[:], d["hidT"][k * P : (k + 1) * P, :])
                    hidT.append(t)

                # embedding rows -> exT k-tiles [128, 64] via PE transpose
                ex = att_big.tile([B, EMB], F32, tag="ex")
                nc.gpsimd.indirect_dma_start(
                    out=ex[:],
                    out_offset=None,
                    in_=d["emb"][:],
                    in_offset=bass.IndirectOffsetOnAxis(ap=idx[:, :1], axis=0),
                )
                # ---- w1h = hidden @ W1 + b1 (batch-major [8, 1024]) ----
                b1s = small.tile([1, H], F32R, tag="b1s")
                nc.sync.dma_start(b1s[:], d["b1"][:])
                ph_lo = ps_a.tile([BL, 512], F32, space="PSUM", tag="w1h_lo",
                                  bufs=1)
                ph_hi = ps_a.tile([BL, 512], F32, space="PSUM", tag="w1h_hi",
                                  bufs=1)
                for k in range(KT):
                    w1k = stream.tile([P, H], F32R, tag="w1k", bufs=2)
                    nc.sync.dma_start(w1k[:], d["W1"][k * P : (k + 1) * P, :])
                    nc.tensor.matmul(ph_lo[:], hidT[k][:], w1k[:, :512],
                                     start=(k == 0), stop=False)
                    nc.tensor.matmul(ph_hi[:], hidT[k][:], w1k[:, 512:],
                                     start=(k == 0), stop=False)
                nc.tensor.matmul(ph_lo[:], ones[:, :BL], b1s[:, :512],
                                 start=False, stop=True)
                nc.tensor.matmul(ph_hi[:], ones[:, :BL], b1s[:, 512:],
                                 start=False, stop=True)
                w1h = small.tile([BL, H], F32, tag="w1h")
                nc.vector.tensor_copy(w1h[:, :512], ph_lo[:])
                nc.vector.tensor_copy(w1h[:, 512:], ph_hi[:])

                # ---- w1hT m-tiles [128, 8] (+ b2 per-feature) ----
                b2s = small.tile([P, KT], F32, tag="b2s")
                nc.sync.dma_start(b2s[:], d["b2t"][:])
                w1hT = []
                for m in range(KT):
                    pt = ps_a.tile([P, BL], F32, space="PSUM", tag="tr8")
                    nc.tensor.transpose(
                        pt[:], w1h[:, m * P : (m + 1) * P], ident[:BL, :BL]
                    )
                    t = small.tile([P, BL], F32, name=f"w1hT{m}",
                                   tag=f"w1hT{m}")
                    nc.vector.tensor_tensor(
                        out=t[:],
                        in0=pt[:],
                        in1=b2s[:, m : m + 1].to_broadcast((P, BL)),
                        op=ALU.add,
                    )
                    w1hT.append(t)

            # ---- w2e (feature-major) + tanh + score ----
            with tc.tile_pool(name="ps_b", bufs=2, space="PSUM") as ps_b:
                vw = small.tile([P, KT], F32R, tag="vw")
                nc.sync.dma_start(
                    vw[:], d["Vw"][:, 0:1].rearrange("(m p) o -> (p o) m", p=P)
                )
                ps_sc_lo = ps_b.tile([1, 512], F32, space="PSUM", tag="sc_lo",
                                     bufs=1)
                ps_sc_hi = ps_b.tile([1, 512], F32, space="PSUM", tag="sc_hi",
                                     bufs=1)
                with tc.tile_pool(name="tm_pool", bufs=2) as tm_pool:
                    for m in range(KT):
                        # one m-block of W2 as a [128, (kt, m')] tile
                        w2m = tm_pool.tile([P, H], F32R, tag="w2m", bufs=2)
                        emit_fcw(2, upto=FCW_BUFS)
                        nc.sync.dma_start(
                            w2m[:].rearrange("p (kt mm) -> p kt mm", kt=KT),
                            d["W2"][:, m * P : (m + 1) * P].rearrange(
                                "(kt kp) mm -> kp kt mm", kp=P
                            ),
                        )
                        p_lo = ps_b.tile([P, 512], F32, space="PSUM",
                                         tag="w2e_lo")
                        p_hi = ps_b.tile([P, 512], F32, space="PSUM",
                                         tag="w2e_hi")
                        for k in range(KT):
                            lhs = w2m[:, k * P : (k + 1) * P]
                            nc.tensor.matmul(
                                p_lo[:], lhs, encT[k][:, :512],
                                start=(k == 0), stop=(k == KT - 1),
                            )
                            nc.tensor.matmul(
                                p_hi[:], lhs, encT[k][:, 512:],
                                start=(k == 0), stop=(k == KT - 1),
                            )
                        tma = tm_pool.tile([P, BL * S], F32, tag="tma")
                        tm = tm_pool.tile([P, BL * S], F32R, tag="tm")
                        bc = w1hT[m][:].rearrange(
                            "p (b o) -> p b o", o=1
                        ).to_broadcast((P, BL, S))
                        nc.vector.tensor_tensor(
                            out=tma[:, :512].rearrange(
                                "p (b s) -> p b s", s=S),
                            in0=p_lo[:].rearrange("p (b s) -> p b s", s=S),
                            in1=bc[:, :BL // 2, :],
                            op=ALU.add,
                        )
                        nc.vector.tensor_tensor(
                            out=tma[:, 512:].rearrange(
                                "p (b s) -> p b s", s=S),
                            in0=p_hi[:].rearrange("p (b s) -> p b s", s=S),
                            in1=bc[:, BL // 2 :, :],
                            op=ALU.add,
                        )
                        nc.scalar.activation(tm[:], tma[:], AF.Tanh)
                        nc.tensor.matmul(
                            ps_sc_lo[:], vw[:, m : m + 1], tm[:, :512],
                            start=(m == 0), stop=(m == KT - 1),
                        )
                        nc.tensor.matmul(
                            ps_sc_hi[:], vw[:, m : m + 1], tm[:, 512:],
                            start=(m == 0), stop=(m == KT - 1),
                        )

                # ---- softmax over S (per batch) ----
                sc_row = small.tile([1, BL * S], F32, tag="sc_row")
                nc.vector.tensor_copy(sc_row[:, :512], ps_sc_lo[:])
                nc.vector.tensor_copy(sc_row[:, 512:], ps_sc_hi[:])

                # embedding rows -> exT k-tiles (PE is idle during softmax)
                exT = []
                for k in range(KT):
                    pt = ps_b.tile([P, B], F32, space="PSUM", tag="trB")
                    nc.tensor.transpose(
                        pt[:], ex[:, k * P : (k + 1) * P], ident[:B, :B]
                    )
                    t = small.tile([P, B], F32R, name=f"exT{k}",
                                   tag=f"exT{k}")
                    nc.vector.tensor_copy(t[:], pt[:])
                    exT.append(t)
                emit_fcw(4, upto=FCW_BUFS)

            sc_dram = dram.tile([1, BL * S], F32, tag="sc_dram")
            nc.sync.dma_start(sc_dram[:], sc_row[:])
            s8 = small.tile([BL, S], F32, tag="s8")
            nc.sync.dma_start(
                s8[:], sc_dram[0:1, :].rearrange("o (b s) -> (o b) s", b=BL)
            )
            mx = small.tile([BL, 1], F32, tag="mx")
            nc.vector.reduce_max(mx[:], s8[:], axis=AX.X)
            nmx = small.tile([BL, 1], F32, tag="nmx")
            nc.vector.tensor_scalar_mul(nmx[:], mx[:], -1.0)
            e8 = small.tile([BL, S], F32, tag="e8")
            ssum = small.tile([BL, 1], F32, tag="ssum")
            nc.scalar.activation(e8[:], s8[:], AF.Exp, bias=nmx[:],
                                 accum_out=ssum[:])
            rsum = small.tile([BL, 1], F32, tag="rsum")
            nc.vector.reciprocal(rsum[:], ssum[:])
            w8 = small.tile([BL, S], F32, tag="w8")
            nc.vector.tensor_scalar_mul(w8[:], e8[:], rsum[:])
            nc.sync.dma_start(o_attw[:], w8[:])

            # ---- context via DVE on encT: ctxT[f, b] = sum_s encT*w ----
            w_dram = dram.tile([1, BL * S], F32, tag="w_dram")
            nc.sync.dma_start(
                w_dram[0:1, :].rearrange("o (b s) -> (o b) s", b=BL), w8[:]
            )
            w128 = big.tile([P, BL * S], F32, tag="w128")
            nc.sync.dma_start(
                w128[:], w_dram[0:1, :].to_broadcast((P, BL * S))
            )
            cc_ctx_in = dram.tile([H, BL], F32R, tag="cc_ctx_in")
            cc_ctx_out = dram.tile([NCORES, H, BL], F32R, tag="cc_ctx_out")
            for f in range(KT):
                prod = big.tile([P, BL * S], F32, tag="prod", bufs=1)
                nc.vector.tensor_tensor(
                    out=prod[:], in0=encT[f][:].bitcast(F32), in1=w128[:],
                    op=ALU.mult,
                )
                ct = small.tile([P, BL], F32R, tag="ctxT", bufs=2)
                with nc.allow_low_precision(reason="f32r is full-width"):
                    nc.vector.reduce_sum(
                        ct[:], prod[:].rearrange("p (b s) -> p b s", s=S),
                        axis=AX.X,
                    )
                nc.sync.dma_start(cc_ctx_in[f * P : (f + 1) * P, :], ct[:])
            att_big_cm.__exit__(None, None, None)
            nc.gpsimd.collective_compute(
                "AllGather", ALU.bypass, replica_groups=RG,
                ins=[cc_ctx_in[:].opt()], outs=[cc_ctx_out[:].opt()],
            )
            xt_ctx = []
            for k in range(KT):
                t = small.tile([P, B], F32R, name=f"xtctx{k}", tag=f"xtctx{k}")
                nc.sync.dma_start(
                    t[:].rearrange("p (c j) -> p c j", c=NCORES),
                    cc_ctx_out[:, k * P : (k + 1) * P, :].transpose([1, 0, 2]),
                )
                xt_ctx.append(t)

            # ---- LSTM stack (gate-column sharded) ----
            prevT = exT
            with tc.tile_pool(name="ps_d", bufs=2, space="PSUM") as ps_d:
                warm(ps_d, 4)
                for l in range(1, 5):
                    bls = small.tile([1, 3 * HL], F32R, tag="bls", bufs=2)
                    nc.sync.dma_start(bls[:], d[f"bl{l}"][:])
                    pg = ps_d.tile([B, 3 * HL], F32, space="PSUM", tag="lstm")
                    xt_all = xt_ctx + prevT
                    # the half that doesn't depend on the newest AllGather
                    # goes first, so the PE chews on it during the gather
                    ks = (list(range(KT, 2 * KT)) + list(range(KT))
                          if l == 1 else list(range(2 * KT)))
                    emit_fcw(3, upto=FCW_BUFS)
                    for j, k in enumerate(ks):
                        wkt = stream.tile([P, 3 * HL], F32R, tag="wkt", bufs=6)
                        nc.sync.dma_start(
                            wkt[:], d[f"Wk{l}"][k * P : (k + 1) * P, :]
                        )
                        nc.tensor.matmul(pg[:], xt_all[k][:], wkt[:],
                                         start=(j == 0), stop=False)
                    nc.tensor.matmul(pg[:], ones[:], bls[:],
                                     start=False, stop=True)
                    ci = small.tile([B, HL], F32, tag="ci")
                    nc.scalar.activation(ci[:], pg[:, :HL], AF.Sigmoid)
                    tg = small.tile([B, HL], F32, tag="tg")
                    nc.scalar.activation(tg[:], pg[:, HL : 2 * HL], AF.Tanh)
                    cst = small.tile([B, HL], F32, tag="cst")
                    nc.vector.tensor_tensor(out=cst[:], in0=ci[:], in1=tg[:],
                                            op=ALU.mult)
                    tc2 = small.tile([B, HL], F32, tag="tc2")
                    nc.scalar.activation(tc2[:], cst[:], AF.Tanh)
                    so = small.tile([B, HL], F32, tag="so")
                    nc.scalar.activation(so[:], pg[:, 2 * HL :], AF.Sigmoid)
                    hsb = small.tile([B, HL], F32, tag="hsb")
                    nc.vector.tensor_tensor(out=hsb[:], in0=so[:], in1=tc2[:],
                                            op=ALU.mult)
                    if l == 4:
                        nc.sync.dma_start(o_h4[:], hsb[:])
                    pt = ps_d.tile([HL, B], F32, space="PSUM", tag="trB")
                    nc.tensor.transpose(pt[:], hsb[:], ident[:B, :B])
                    hT_chunk = small.tile([HL, B], F32R, tag="hT_chunk",
                                          bufs=2)
                    nc.vector.tensor_copy(hT_chunk[:], pt[:])
                    cc_h_in = dram.tile([HL, B], F32R, tag=f"cc_h_in{l}")
                    cc_h_out = dram.tile([H, B], F32R, tag=f"cc_h_out{l}")
                    nc.sync.dma_start(cc_h_in[:], hT_chunk[:])
                    nc.gpsimd.collective_compute(
                        "AllGather", ALU.bypass, replica_groups=RG,
                        ins=[cc_h_in[:].opt()], outs=[cc_h_out[:].opt()],
                    )
                    warm(ps_d, 4)
                    newT = []
                    for k in range(KT):
                        t = small.tile([P, B], F32R, name=f"hT{l}_{k}",
                                       tag=f"hT{l % 2}_{k}")
                        nc.sync.dma_start(
                            t[:], cc_h_out[k * P : (k + 1) * P, :]
                        )
                        newT.append(t)
                    prevT = newT

            # ---- fc: logits = h4 @ fcW + fcb (vocab-sharded, k-outer) ----
            with tc.tile_pool(name="ps_e", bufs=1, space="PSUM") as ps_e:
                emit_fcw(64, upto=KT)
                pf = [
                    ps_e.tile([B, NW], F32, space="PSUM", tag=f"fc{n}",
                              name=f"fc{n}")
                    for n in range(NT)
                ]
                for k in range(KT):
                    for n in range(NT):
                        nc.tensor.matmul(
                            pf[n][:],
                            prevT[k][:],
                            fcw_tiles[k][:, n * NW : (n + 1) * NW],
                            start=(k == 0), stop=False,
                        )
                for n in range(NT):
                    fcbn = stream.tile([1, NW], F32R, tag="fcbn", bufs=2)
                    nc.sync.dma_start(
                        fcbn[:], d["fcb"][0:1, n * NW : (n + 1) * NW]
                    )
                    nc.tensor.matmul(pf[n][:], ones[:], fcbn[:],
                                     start=False, stop=True)
                    lg = small.tile([B, NW], F32, tag="lg", bufs=2)
                    nc.vector.tensor_copy(lg[:], pf[n][:])
                    nc.sync.dma_start(
                        o_logits[:, n * NW : (n + 1) * NW], lg[:]
                    )

    nc.finalize()
    return nc


_NC = None


def _get_nc():
    global _NC
    if _NC is None:
        _NC = _build()
    return _NC


def _prep_in_maps(inputs):
    return _shard(**{k: np.asarray(v) for k, v in inputs.items()})


def _shard(x, hidden, enc_output, W1, b1, W2, b2, Vw, Vb, emb,
           Wk1, Wr1, bl1, Wk2, Wr2, bl2, Wk3, Wr3, bl3, Wk4, Wr4, bl4,
           fcW, fcb):
    f32 = np.float32
    x = np.ascontiguousarray(np.asarray(x).astype(np.int32).reshape(B, 1))
    hidden = np.asarray(hidden, f32)
    enc = np.asarray(enc_output, f32).reshape(B, S, H)
    W1 = np.ascontiguousarray(np.asarray(W1, f32))
    W2 = np.ascontiguousarray(np.asarray(W2, f32))
    b1 = np.asarray(b1, f32).reshape(1, H)
    b2t = np.ascontiguousarray(np.asarray(b2, f32).reshape(KT, P).T)
    Vw = np.ascontiguousarray(np.asarray(Vw, f32).reshape(H, 1))
    emb = np.ascontiguousarray(np.asarray(emb, f32))
    fcW = np.ascontiguousarray(np.asarray(fcW, f32))
    fcb = np.asarray(fcb, f32).reshape(1, VOCAB)
    ones = np.ones((1, B), f32)
    Wks = [np.asarray(w, f32) for w in (Wk1, Wk2, Wk3, Wk4)]
    bls = [np.asarray(v, f32).reshape(4 * H) for v in (bl1, bl2, bl3, bl4)]

    in_maps = []
    for c in range(NCORES):
        bsl = slice(c * BL, (c + 1) * BL)
        csl = [slice(g * H + c * HL, g * H + (c + 1) * HL) for g in (0, 2, 3)]
        m = {
            "x_idx": x,
            "hidT": np.ascontiguousarray(hidden[bsl].T),
            "encT": np.ascontiguousarray(enc[bsl].reshape(BL * S, H).T),
            "W1": W1, "b1": b1, "W2": W2, "b2t": b2t, "Vw": Vw, "emb": emb,
            "fcW": np.ascontiguousarray(fcW[:, c * VL : (c + 1) * VL]),
            "fcb": np.ascontiguousarray(fcb[:, c * VL : (c + 1) * VL]),
            "ones": ones,
        }
        for l in range(4):
            m[f"Wk{l + 1}"] = np.ascontiguousarray(
                np.concatenate([Wks[l][:, s] for s in csl], axis=1)
            )
            m[f"bl{l + 1}"] = np.ascontiguousarray(
                np.concatenate([bls[l][s] for s in csl]).reshape(1, 3 * HL)
            )
        in_maps.append(m)
    return in_maps


def kernel(**inputs):
    nc = _get_nc()
    in_maps = _prep_in_maps(inputs)
    res = run_bass_kernel_spmd(nc, in_maps, core_ids=list(range(NCORES)))
    outs = res.results
    logits = np.concatenate([outs[c]["logits"] for c in range(NCORES)], axis=1)
    h4 = np.concatenate([outs[c]["h4"] for c in range(NCORES)], axis=1)
    attw = np.concatenate([outs[c]["attw"] for c in range(NCORES)], axis=0)
    return logits, h4, attw.reshape(B, S, 1)
